# revision 21
# baseline (speedup 1.0000x reference)
"""Multi-head attention (B=8, S=1024, C=768, H=12, Dh=64) on 8 TRN2 NeuronCores.

Strategy: pure data parallelism — batch element i runs on core i. Per core:
  qkvT = w_qkv @ x^T      (o-major layout; q,k kept transposed [d, s])
  v    = x @ w_qkv_v^T    (t-major layout, augmented with a ones column)
  scoresT[t,s] = k q^T    -> exp (no max-subtract; logits are small)
  outU^T = [v | 1]^T @ att  (row 64 of PSUM = softmax denominators)
  outT = outU^T / sums;  y^T = w_proj @ outT + b
All matmuls run in bf16 (fp32 PSUM accumulation). Host pre-transposes
x / weights so no on-chip transposes are needed; host converts to bf16.
"""
import os

import numpy as np
import ml_dtypes

import concourse.bass as bass
import concourse.tile as tile
from concourse import bacc, mybir
from concourse.bass_utils import run_bass_kernel_spmd

N_CORES = 8
S, C, H, Dh = 1024, 768, 12, 64
NB_C = C // 128          # 6 c-blocks
NB_S = S // 128          # 8 s/t-blocks
NJ = S // 512            # 2 free-dim column blocks
f32 = mybir.dt.float32
bf16 = mybir.dt.bfloat16
BF = ml_dtypes.bfloat16

TRACE = False
LAST_RESULT = None

_cache = {}


def _build():
    nc = bacc.Bacc("TRN2", target_bir_lowering=False, debug=False, num_devices=N_CORES)
    xT_d = nc.dram_tensor("xT", [NB_C, 128, S], bf16, kind="ExternalInput").ap()
    wq_d = nc.dram_tensor("wqkvT", [NB_C, 128, 3 * C], bf16, kind="ExternalInput").ap()
    wp_d = nc.dram_tensor("wprojT", [NB_C, 128, C], bf16, kind="ExternalInput").ap()
    bias_d = nc.dram_tensor("bias", [128, NB_C], f32, kind="ExternalInput").ap()
    out_d = nc.dram_tensor("out", [NB_C, 128, S], f32, kind="ExternalOutput").ap()

    Exp = mybir.ActivationFunctionType.Exp

    with tile.TileContext(nc) as tc:
        with (
            tc.tile_pool(name="singles", bufs=1) as singles,
            tc.tile_pool(name="att", bufs=16) as att_pool,
            tc.tile_pool(name="inv", bufs=4) as inv_pool,
            tc.tile_pool(name="binv", bufs=4) as binv_pool,
            tc.tile_pool(name="ya", bufs=12) as ya_pool,
            tc.tile_pool(name="y", bufs=3) as y_pool,
            tc.tile_pool(name="qkv_ps", bufs=2, space="PSUM") as qkv_ps,
            tc.tile_pool(name="sc_ps", bufs=2, space="PSUM") as sc_ps,
            tc.tile_pool(name="av_ps", bufs=2, space="PSUM") as av_ps,
        ):
            xT = singles.tile([128, NB_C, S], bf16)
            wq = singles.tile([128, NB_C, 3 * C], bf16)
            wp = singles.tile([128, NB_C, C], bf16)
            bias = singles.tile([128, NB_C], f32)
            # DMA order = dependency order: xT, q/k pair 0, v, pairs 1-5, wp,
            # bias. The first transfers are chunked ~64KB so they spread over
            # all 16 DMA queues (per-queue bandwidth is ~1/16 of HBM).
            for cb in range(NB_C):
                for q4 in range(4):
                    nc.sync.dma_start(
                        out=xT[:, cb, q4 * 256:(q4 + 1) * 256],
                        in_=xT_d[cb, :, q4 * 256:(q4 + 1) * 256],
                    )
            for cb in range(NB_C):
                nc.sync.dma_start(out=wq[:, cb, 0:256], in_=wq_d[cb, :, 0:256])
            for cb in range(NB_C):
                for h2 in range(2):
                    nc.sync.dma_start(
                        out=wq[:, cb, 2 * C + h2 * 384:2 * C + (h2 + 1) * 384],
                        in_=wq_d[cb, :, 2 * C + h2 * 384:2 * C + (h2 + 1) * 384],
                    )
            for p in range(1, 6):
                for cb in range(NB_C):
                    nc.sync.dma_start(
                        out=wq[:, cb, p * 256:(p + 1) * 256],
                        in_=wq_d[cb, :, p * 256:(p + 1) * 256],
                    )
            for cb in range(NB_C):
                nc.sync.dma_start(out=wp[:, cb, :], in_=wp_d[cb])
            nc.sync.dma_start(out=bias[:], in_=bias_d[:])

            qkT = singles.tile([128, 2 * NB_C, S], bf16)   # q at idx p, k at idx 6+p
            vaug = singles.tile([128, NB_S, H, 65], bf16)  # [t, tb, h, d|1]
            outUT = singles.tile([128, NB_C, S], bf16)

            # PE warmup during the DMA fill: dummy matmuls with NO deps at all
            # (uninitialized SBUF is fine - the psum result is never read).
            # PE starts the moment the engine comes up and un-throttles the
            # HAM clock gate before the first real matmul.
            warm = singles.tile([128, 512], bf16)
            nc.vector.memset(warm[:], 0.0)
            for wi in range(1):
                ps_w = qkv_ps.tile([128, 512], f32, tag="qkvps")
                for wj in range(10):
                    nc.tensor.matmul(ps_w[:], warm[:, 0:128], warm[:], start=True, stop=True)

            # ones column of v_aug: memset the WHOLE tile to 1.0 (contiguous,
            # fast, on the idle gpsimd) - A-v copies overwrite cols 0-63 later.
            nc.gpsimd.memset(vaug[:], 1.0)

            # ---------- emission helpers ----------
            def emit_aqk(p, which, j):
                # q/k projection: head-pair p, which 0=q 1=k, s-column block j
                ps_qk = qkv_ps.tile([128, 512], f32, tag="qkvps")
                col0 = p * 256 + which * 128
                for cb in range(NB_C):
                    nc.tensor.matmul(
                        ps_qk[:],
                        wq[:, cb, col0:col0 + 128],
                        xT[:, cb, j * 512:(j + 1) * 512],
                        start=(cb == 0), stop=(cb == NB_C - 1),
                    )
                nc.vector.tensor_copy(
                    qkT[:, which * NB_C + p, j * 512:(j + 1) * 512], ps_qk[:]
                )

            def emit_av_stage(tb, half):
                # v projection into the augmented [t, h, d|1] tile
                ps_v = qkv_ps.tile([128, 512], f32, tag="qkvps")
                for cb in range(NB_C):
                    nc.tensor.matmul(
                        ps_v[:, 0:384],
                        xT[:, cb, tb * 128:(tb + 1) * 128],
                        wq[:, cb, 2 * C + half * 384:2 * C + (half + 1) * 384],
                        start=(cb == 0), stop=(cb == NB_C - 1),
                    )
                nc.vector.tensor_copy(
                    vaug[:, tb, half * 6:(half + 1) * 6, 0:64],
                    ps_v[:, 0:384].rearrange("p (h d) -> p h d", d=64),
                )

            atts = [[None] * NB_S for _ in range(H)]
            pavs = [[None] * NJ for _ in range(H)]
            sums_t = [None] * H

            def emit_qk(h, tb):
                hp = 64 * (h % 2)
                p = h // 2
                qT = qkT[hp:hp + 64, p, :]
                kT = qkT[hp:hp + 64, NB_C + p, :]
                att = att_pool.tile([128, S], bf16, tag="att")
                atts[h][tb] = att
                sc = sc_ps.tile([128, 1024], f32, tag="scps")
                for j in range(NJ):
                    nc.tensor.matmul(
                        sc[:, j * 512:(j + 1) * 512],
                        kT[:, tb * 128:(tb + 1) * 128],
                        qT[:, j * 512:(j + 1) * 512],
                        start=True, stop=True,
                    )
                nc.scalar.activation(att[:], sc[:], Exp, scale=0.125)

            def emit_av(h, j):
                if sums_t[h] is None:
                    sums_t[h] = inv_pool.tile([1, S], f32, tag="sums", name=f"sums{h}")
                pav = av_ps.tile([65, 512], f32, tag="avps")
                pavs[h][j] = pav
                for tb in range(NB_S):
                    nc.tensor.matmul(
                        pav[:],
                        vaug[:, tb, h, :],
                        atts[h][tb][:, j * 512:(j + 1) * 512],
                        start=(tb == 0), stop=(tb == NB_S - 1),
                    )
                nc.vector.tensor_copy(
                    sums_t[h][:, j * 512:(j + 1) * 512], pav[64:65, :]
                )

            def emit_norm_j(h, j):
                hp = 64 * (h % 2)
                inv = inv_pool.tile([1, 512], f32, tag="inv", name=f"invj{h}_{j}")
                nc.vector.reciprocal_approx_fast(
                    inv[:], sums_t[h][:, j * 512:(j + 1) * 512]
                )
                binv = binv_pool.tile([128, 512], f32, tag="binv", name=f"binvj{h}_{j}")
                nc.gpsimd.partition_broadcast(binv[:], inv[:])
                nc.vector.tensor_mul(
                    outUT[hp:hp + 64, h // 2, j * 512:(j + 1) * 512],
                    pavs[h][j][0:64, :],
                    binv[hp:hp + 64, :],
                )

            def emit_norm(h):
                hp = 64 * (h % 2)
                inv = inv_pool.tile([1, S], f32, tag="inv")
                nc.vector.reciprocal_approx_fast(inv[:], sums_t[h][:])
                binv = binv_pool.tile([128, S], f32, tag="binv")
                nc.gpsimd.partition_broadcast(binv[:], inv[:])
                for j in range(NJ):
                    nc.vector.tensor_mul(
                        outUT[hp:hp + 64, h // 2, j * 512:(j + 1) * 512],
                        pavs[h][j][0:64, :],
                        binv[hp:hp + 64, j * 512:(j + 1) * 512],
                    )

            ya_t = {}

            def emit_proj_a(cb, j):
                # first half-accumulation (kb 0-2 = heads 0-5) + bias
                pp = qkv_ps.tile([128, 512], f32, tag="qkvps")
                for kb in range(3):
                    nc.tensor.matmul(
                        pp[:],
                        wp[:, kb, cb * 128:(cb + 1) * 128],
                        outUT[:, kb, j * 512:(j + 1) * 512],
                        start=(kb == 0), stop=(kb == 2),
                    )
                ya = ya_pool.tile([128, 512], f32, tag="ya")
                ya_t[(cb, j)] = ya
                nc.vector.tensor_scalar_add(ya[:], pp[:], bias[:, cb:cb + 1])

            def emit_proj_b(cb, j):
                # second partial: kb 3-4 (heads 6-9), accumulate into ya
                pp = qkv_ps.tile([128, 512], f32, tag="qkvps")
                for kb in range(3, 5):
                    nc.tensor.matmul(
                        pp[:],
                        wp[:, kb, cb * 128:(cb + 1) * 128],
                        outUT[:, kb, j * 512:(j + 1) * 512],
                        start=(kb == 3), stop=(kb == 4),
                    )
                nc.vector.tensor_add(ya_t[(cb, j)][:], pp[:], ya_t[(cb, j)][:])

            def emit_proj_c(cb, j, alt=False):
                # final partial: kb 5 (heads 10-11) + combine + store
                if alt:
                    pp_t = sc_ps.tile([128, 1024], f32, tag="scps", name=f"ppc{cb}_{j}")
                    pp = pp_t[:, 0:512]
                else:
                    pp = qkv_ps.tile([128, 512], f32, tag="qkvps")
                nc.tensor.matmul(
                    pp[:],
                    wp[:, 5, cb * 128:(cb + 1) * 128],
                    outUT[:, 5, j * 512:(j + 1) * 512],
                    start=True, stop=True,
                )
                y = y_pool.tile([128, 512], f32, tag="y")
                nc.vector.tensor_add(y[:], pp[:] if not alt else pp, ya_t[(cb, j)][:])
                nc.sync.dma_start(out=out_d[cb, :, j * 512:(j + 1) * 512], in_=y[:])

            # ---------- emission order ----------
            # bootstrap: pair 0 q/k, then head-0 QKs interleaved with A-v half 0
            for j in range(NJ):
                emit_aqk(0, 0, j)
            for j in range(NJ):
                emit_aqk(0, 1, j)
            emit_qk(0, 0)
            emit_qk(0, 1)
            counts = [2, 2, 1, 1, 1, 1]
            ui = 0
            for i, tb in enumerate(range(2, NB_S)):
                for _ in range(counts[i]):
                    emit_av_stage(ui, 0)
                    ui += 1
                emit_qk(0, tb)

            # filler units per head period: A-v half1, remaining q/k pairs,
            # then the first two projection partials
            fillers = {
                1: [("aqk", 1), ("av", 0), ("av", 1)],
                2: [("aqk", 2), ("av", 2), ("av", 3)],
                3: [("aqk", 3), ("av", 4), ("av", 5)],
                4: [("av", 6), ("av", 7)],
                5: [("aqk", 4)],
                6: [("pa", 0), ("pa", 1), ("pa", 2), ("pa", 3)],
                7: [("aqk", 5)],
                8: [("pa", 4), ("pa", 5), ("pa", 6), ("pa", 7)],
                9: [("pa", 8), ("pa", 9), ("pa", 10), ("pa", 11)],
                10: [("pb", 0), ("pb", 1), ("pb", 2), ("pb", 3), ("pb", 4)],
                11: [("pb", 5), ("pb", 6), ("pb", 7), ("pb", 8)],
            }
            cj_units = [(cb, j) for cb in range(NB_C) for j in range(NJ)]

            def emit_filler(kind, a):
                if kind == "aqk":
                    for j in range(NJ):
                        emit_aqk(a, 0, j)
                        emit_aqk(a, 1, j)
                elif kind == "av":
                    # A-v half 1, tb index a (half 0 done in bootstrap)
                    emit_av_stage(a, 1)
                elif kind == "pa":
                    emit_proj_a(*cj_units[a])
                elif kind == "pb":
                    emit_proj_b(*cj_units[a])

            for h in range(1, H):
                fl = list(fillers.get(h, []))
                emit_qk(h, 0)
                emit_qk(h, 1)
                emit_av(h - 1, 0)
                emit_qk(h, 2)
                if fl and fl[0][0] in ("aqk", "av"):
                    emit_filler(*fl.pop(0))
                emit_qk(h, 3)
                emit_av(h - 1, 1)
                emit_norm(h - 1)
                emit_qk(h, 4)
                if fl:
                    emit_filler(*fl.pop(0))
                emit_qk(h, 5)
                emit_qk(h, 6)
                for u in fl:
                    emit_filler(*u)
                emit_qk(h, 7)
            emit_av(H - 1, 0)
            emit_norm_j(H - 1, 0)
            emit_proj_b(*cj_units[9])
            emit_av(H - 1, 1)
            emit_norm_j(H - 1, 1)
            emit_proj_b(*cj_units[10])
            emit_proj_b(*cj_units[11])
            for j in range(NJ):
                for cb in range(NB_C):
                    emit_proj_c(cb, j, alt=(cb % 2 == 1))

    nc.compile()
    return nc


def _patch_ldw_opt():
    # experiment: let walrus hoist LDWEIGHTS (default pipeline disables it)
    import concourse.bass_utils as bu

    if getattr(bu, "_ldw_patched", False):
        return
    orig = bu.run_command

    def patched(cmd, *a, **kw):
        cmd = [
            c.replace("--enable-ldw-opt=false", "--enable-ldw-opt=true")
            if isinstance(c, str) else c
            for c in cmd
        ]
        return orig(cmd, *a, **kw)

    bu.run_command = patched
    bu._ldw_patched = True


def kernel(x, w_qkv, w_proj, b_proj):
    global LAST_RESULT
    if os.environ.get("LDW_OPT") == "1":
        _patch_ldw_opt()
    if "nc" not in _cache:
        _cache["nc"] = _build()
    nc = _cache["nc"]

    # permute qkv output columns to [q_p | k_p] head-pair-interleaved, v last
    wqkvT_full = w_qkv.astype(np.float32).T  # [c, o]
    cols = []
    for p in range(NB_C):
        cols.append(wqkvT_full[:, p * 128:(p + 1) * 128])          # q pair p
        cols.append(wqkvT_full[:, C + p * 128:C + (p + 1) * 128])  # k pair p
    cols.append(wqkvT_full[:, 2 * C:3 * C])                        # v
    wqkvT = np.ascontiguousarray(
        np.concatenate(cols, axis=1).astype(BF).reshape(NB_C, 128, 3 * C)
    )
    wprojT = np.ascontiguousarray(
        w_proj.astype(np.float32).T.astype(BF).reshape(NB_C, 128, C)
    )
    bias = np.ascontiguousarray(b_proj.astype(np.float32).reshape(NB_C, 128).T)
    in_maps = []
    for i in range(N_CORES):
        xT = np.ascontiguousarray(
            x[i].astype(np.float32).T.astype(BF).reshape(NB_C, 128, S)
        )
        in_maps.append({"xT": xT, "wqkvT": wqkvT, "wprojT": wprojT, "bias": bias})

    res = run_bass_kernel_spmd(
        nc, in_maps, core_ids=list(range(N_CORES)), trace=TRACE
    )
    LAST_RESULT = res

    out = np.empty((N_CORES, S, C), np.float32)
    for i in range(N_CORES):
        out[i] = res.results[i]["out"].reshape(C, S).T
    return out


# revision 23
# speedup vs baseline: 1.0486x; 1.0486x over previous
"""Multi-head attention (B=8, S=1024, C=768, H=12, Dh=64) on 8 TRN2 NeuronCores.

Strategy: pure data parallelism — batch element i runs on core i. Per core:
  qkvT = w_qkv @ x^T      (o-major layout; q,k kept transposed [d, s])
  v    = x @ w_qkv_v^T    (t-major layout, augmented with a ones column)
  scoresT[t,s] = k q^T    -> exp (no max-subtract; logits are small)
  outU^T = [v | 1]^T @ att  (row 64 of PSUM = softmax denominators)
  outT = outU^T / sums;  y^T = w_proj @ outT + b
All matmuls run in bf16 (fp32 PSUM accumulation). Host pre-transposes
x / weights so no on-chip transposes are needed; host converts to bf16.
"""
import os

import numpy as np
import ml_dtypes

import concourse.bass as bass
import concourse.tile as tile
from concourse import bacc, mybir
from concourse.bass_utils import run_bass_kernel_spmd

N_CORES = 8
S, C, H, Dh = 1024, 768, 12, 64
NB_C = C // 128          # 6 c-blocks
NB_S = S // 128          # 8 s/t-blocks
NJ = S // 512            # 2 free-dim column blocks
f32 = mybir.dt.float32
bf16 = mybir.dt.bfloat16
BF = ml_dtypes.bfloat16

TRACE = False
LAST_RESULT = None

_cache = {}


def _build():
    nc = bacc.Bacc("TRN2", target_bir_lowering=False, debug=False, num_devices=N_CORES)
    xT_d = nc.dram_tensor("xT", [128, NB_C, S], bf16, kind="ExternalInput").ap()
    wq_d = nc.dram_tensor("wqkvT", [128, 9, NB_C, 256], bf16, kind="ExternalInput").ap()
    wp_d = nc.dram_tensor("wprojT", [128, NB_C, C], bf16, kind="ExternalInput").ap()
    bias_d = nc.dram_tensor("bias", [128, NB_C], f32, kind="ExternalInput").ap()
    out_d = nc.dram_tensor("out", [NB_C, 128, S], f32, kind="ExternalOutput").ap()

    Exp = mybir.ActivationFunctionType.Exp

    with tile.TileContext(nc) as tc:
        with (
            tc.tile_pool(name="singles", bufs=1) as singles,
            tc.tile_pool(name="att", bufs=16) as att_pool,
            tc.tile_pool(name="inv", bufs=4) as inv_pool,
            tc.tile_pool(name="binv", bufs=4) as binv_pool,
            tc.tile_pool(name="ya", bufs=12) as ya_pool,
            tc.tile_pool(name="y", bufs=3) as y_pool,
            tc.tile_pool(name="qkv_ps", bufs=2, space="PSUM") as qkv_ps,
            tc.tile_pool(name="sc_ps", bufs=2, space="PSUM") as sc_ps,
            tc.tile_pool(name="av_ps", bufs=2, space="PSUM") as av_ps,
        ):
            xT = singles.tile([128, NB_C, S], bf16)
            wq = singles.tile([128, NB_C, 3 * C], bf16)
            wp = singles.tile([128, NB_C, C], bf16)
            bias = singles.tile([128, NB_C], f32)
            # DMA order = dependency order: xT, q/k pair 0, v, pairs 1-5,
            # wp, bias. Host layouts are partition-major so every transfer is
            # ONE dma_start with large per-partition-contiguous descriptors.
            nc.sync.dma_start(out=xT[:], in_=xT_d[:])
            nc.sync.dma_start(out=wq[:, :, 0:256], in_=wq_d[:, 0])
            for c3 in range(3):
                nc.sync.dma_start(
                    out=wq[:, :, 2 * C + c3 * 256:2 * C + (c3 + 1) * 256],
                    in_=wq_d[:, 6 + c3],
                )
            for p in range(1, 6):
                nc.sync.dma_start(out=wq[:, :, p * 256:(p + 1) * 256], in_=wq_d[:, p])
            nc.sync.dma_start(out=wp[:], in_=wp_d[:])
            nc.sync.dma_start(out=bias[:], in_=bias_d[:])

            qkT = singles.tile([128, 2 * NB_C, S], bf16)   # q at idx p, k at idx 6+p
            vaug = singles.tile([128, NB_S, H, 65], bf16)  # [t, tb, h, d|1]
            outUT = singles.tile([128, NB_C, S], bf16)

            # PE warmup during the DMA fill: dummy matmuls with NO deps at all
            # (uninitialized SBUF is fine - the psum result is never read).
            # PE starts the moment the engine comes up and un-throttles the
            # HAM clock gate before the first real matmul.
            warm = singles.tile([128, 512], bf16)
            nc.vector.memset(warm[:], 0.0)
            for wi in range(1):
                ps_w = qkv_ps.tile([128, 512], f32, tag="qkvps")
                for wj in range(10):
                    nc.tensor.matmul(ps_w[:], warm[:, 0:128], warm[:], start=True, stop=True)

            # ones column of v_aug: memset the WHOLE tile to 1.0 (contiguous,
            # fast, on the idle gpsimd) - A-v copies overwrite cols 0-63 later.
            nc.gpsimd.memset(vaug[:], 1.0)

            # ---------- emission helpers ----------
            def emit_aqk(p, which, j):
                # q/k projection: head-pair p, which 0=q 1=k, s-column block j
                ps_qk = qkv_ps.tile([128, 512], f32, tag="qkvps")
                col0 = p * 256 + which * 128
                for cb in range(NB_C):
                    nc.tensor.matmul(
                        ps_qk[:],
                        wq[:, cb, col0:col0 + 128],
                        xT[:, cb, j * 512:(j + 1) * 512],
                        start=(cb == 0), stop=(cb == NB_C - 1),
                    )
                nc.vector.tensor_copy(
                    qkT[:, which * NB_C + p, j * 512:(j + 1) * 512], ps_qk[:]
                )

            def emit_av_stage(tb, half):
                # v projection into the augmented [t, h, d|1] tile
                ps_v = qkv_ps.tile([128, 512], f32, tag="qkvps")
                for cb in range(NB_C):
                    nc.tensor.matmul(
                        ps_v[:, 0:384],
                        xT[:, cb, tb * 128:(tb + 1) * 128],
                        wq[:, cb, 2 * C + half * 384:2 * C + (half + 1) * 384],
                        start=(cb == 0), stop=(cb == NB_C - 1),
                    )
                nc.vector.tensor_copy(
                    vaug[:, tb, half * 6:(half + 1) * 6, 0:64],
                    ps_v[:, 0:384].rearrange("p (h d) -> p h d", d=64),
                )

            atts = [[None] * NB_S for _ in range(H)]
            pavs = [[None] * NJ for _ in range(H)]
            sums_t = [None] * H

            def emit_qk(h, tb):
                hp = 64 * (h % 2)
                p = h // 2
                qT = qkT[hp:hp + 64, p, :]
                kT = qkT[hp:hp + 64, NB_C + p, :]
                att = att_pool.tile([128, S], bf16, tag="att")
                atts[h][tb] = att
                sc = sc_ps.tile([128, 1024], f32, tag="scps")
                for j in range(NJ):
                    nc.tensor.matmul(
                        sc[:, j * 512:(j + 1) * 512],
                        kT[:, tb * 128:(tb + 1) * 128],
                        qT[:, j * 512:(j + 1) * 512],
                        start=True, stop=True,
                    )
                nc.scalar.activation(att[:], sc[:], Exp, scale=0.125)

            def emit_av(h, j):
                if sums_t[h] is None:
                    sums_t[h] = inv_pool.tile([1, S], f32, tag="sums", name=f"sums{h}")
                pav = av_ps.tile([65, 512], f32, tag="avps")
                pavs[h][j] = pav
                for tb in range(NB_S):
                    nc.tensor.matmul(
                        pav[:],
                        vaug[:, tb, h, :],
                        atts[h][tb][:, j * 512:(j + 1) * 512],
                        start=(tb == 0), stop=(tb == NB_S - 1),
                    )
                nc.vector.tensor_copy(
                    sums_t[h][:, j * 512:(j + 1) * 512], pav[64:65, :]
                )

            def emit_norm_j(h, j):
                hp = 64 * (h % 2)
                inv = inv_pool.tile([1, 512], f32, tag="inv", name=f"invj{h}_{j}")
                nc.vector.reciprocal_approx_fast(
                    inv[:], sums_t[h][:, j * 512:(j + 1) * 512]
                )
                binv = binv_pool.tile([128, 512], f32, tag="binv", name=f"binvj{h}_{j}")
                nc.gpsimd.partition_broadcast(binv[:], inv[:])
                nc.vector.tensor_mul(
                    outUT[hp:hp + 64, h // 2, j * 512:(j + 1) * 512],
                    pavs[h][j][0:64, :],
                    binv[hp:hp + 64, :],
                )

            def emit_norm(h):
                hp = 64 * (h % 2)
                inv = inv_pool.tile([1, S], f32, tag="inv")
                nc.vector.reciprocal_approx_fast(inv[:], sums_t[h][:])
                binv = binv_pool.tile([128, S], f32, tag="binv")
                nc.gpsimd.partition_broadcast(binv[:], inv[:])
                for j in range(NJ):
                    nc.vector.tensor_mul(
                        outUT[hp:hp + 64, h // 2, j * 512:(j + 1) * 512],
                        pavs[h][j][0:64, :],
                        binv[hp:hp + 64, j * 512:(j + 1) * 512],
                    )

            ya_t = {}

            def emit_proj_a(cb, j):
                # first half-accumulation (kb 0-2 = heads 0-5) + bias
                pp = qkv_ps.tile([128, 512], f32, tag="qkvps")
                for kb in range(3):
                    nc.tensor.matmul(
                        pp[:],
                        wp[:, kb, cb * 128:(cb + 1) * 128],
                        outUT[:, kb, j * 512:(j + 1) * 512],
                        start=(kb == 0), stop=(kb == 2),
                    )
                ya = ya_pool.tile([128, 512], f32, tag="ya")
                ya_t[(cb, j)] = ya
                nc.vector.tensor_scalar_add(ya[:], pp[:], bias[:, cb:cb + 1])

            def emit_proj_b(cb, j):
                # second partial: kb 3-4 (heads 6-9), accumulate into ya
                pp = qkv_ps.tile([128, 512], f32, tag="qkvps")
                for kb in range(3, 5):
                    nc.tensor.matmul(
                        pp[:],
                        wp[:, kb, cb * 128:(cb + 1) * 128],
                        outUT[:, kb, j * 512:(j + 1) * 512],
                        start=(kb == 3), stop=(kb == 4),
                    )
                nc.vector.tensor_add(ya_t[(cb, j)][:], pp[:], ya_t[(cb, j)][:])

            def emit_proj_c(cb, j, alt=False):
                # final partial: kb 5 (heads 10-11) + combine + store
                if alt:
                    pp_t = sc_ps.tile([128, 1024], f32, tag="scps", name=f"ppc{cb}_{j}")
                    pp = pp_t[:, 0:512]
                else:
                    pp = qkv_ps.tile([128, 512], f32, tag="qkvps")
                nc.tensor.matmul(
                    pp[:],
                    wp[:, 5, cb * 128:(cb + 1) * 128],
                    outUT[:, 5, j * 512:(j + 1) * 512],
                    start=True, stop=True,
                )
                y = y_pool.tile([128, 512], f32, tag="y")
                nc.vector.tensor_add(y[:], pp[:] if not alt else pp, ya_t[(cb, j)][:])
                nc.sync.dma_start(out=out_d[cb, :, j * 512:(j + 1) * 512], in_=y[:])

            # ---------- emission order ----------
            # bootstrap: pair 0 q/k, then head-0 QKs interleaved with A-v half 0
            for j in range(NJ):
                emit_aqk(0, 0, j)
            for j in range(NJ):
                emit_aqk(0, 1, j)
            emit_qk(0, 0)
            emit_qk(0, 1)
            counts = [2, 2, 1, 1, 1, 1]
            ui = 0
            for i, tb in enumerate(range(2, NB_S)):
                for _ in range(counts[i]):
                    emit_av_stage(ui, 0)
                    ui += 1
                emit_qk(0, tb)

            # filler units per head period: A-v half1, remaining q/k pairs,
            # then the first two projection partials
            fillers = {
                1: [("aqk", 1), ("av", 0), ("av", 1)],
                2: [("aqk", 2), ("av", 2), ("av", 3)],
                3: [("aqk", 3), ("av", 4), ("av", 5)],
                4: [("av", 6), ("av", 7)],
                5: [("aqk", 4)],
                6: [("pa", 0), ("pa", 1), ("pa", 2), ("pa", 3)],
                7: [("aqk", 5)],
                8: [("pa", 4), ("pa", 5), ("pa", 6), ("pa", 7)],
                9: [("pa", 8), ("pa", 9), ("pa", 10), ("pa", 11)],
                10: [("pb", 0), ("pb", 1), ("pb", 2), ("pb", 3), ("pb", 4)],
                11: [("pb", 5), ("pb", 6), ("pb", 7), ("pb", 8)],
            }
            cj_units = [(cb, j) for cb in range(NB_C) for j in range(NJ)]

            def emit_filler(kind, a):
                if kind == "aqk":
                    for j in range(NJ):
                        emit_aqk(a, 0, j)
                        emit_aqk(a, 1, j)
                elif kind == "av":
                    # A-v half 1, tb index a (half 0 done in bootstrap)
                    emit_av_stage(a, 1)
                elif kind == "pa":
                    emit_proj_a(*cj_units[a])
                elif kind == "pb":
                    emit_proj_b(*cj_units[a])

            for h in range(1, H):
                fl = list(fillers.get(h, []))
                emit_qk(h, 0)
                emit_qk(h, 1)
                emit_av(h - 1, 0)
                emit_qk(h, 2)
                if fl and fl[0][0] in ("aqk", "av"):
                    emit_filler(*fl.pop(0))
                emit_qk(h, 3)
                emit_av(h - 1, 1)
                emit_norm(h - 1)
                emit_qk(h, 4)
                if fl:
                    emit_filler(*fl.pop(0))
                emit_qk(h, 5)
                emit_qk(h, 6)
                for u in fl:
                    emit_filler(*u)
                emit_qk(h, 7)
            emit_av(H - 1, 0)
            emit_norm_j(H - 1, 0)
            emit_proj_b(*cj_units[9])
            emit_av(H - 1, 1)
            emit_norm_j(H - 1, 1)
            emit_proj_b(*cj_units[10])
            emit_proj_b(*cj_units[11])
            for j in range(NJ):
                for cb in range(NB_C):
                    emit_proj_c(cb, j, alt=(cb % 2 == 1))

    nc.compile()
    return nc


def _patch_ldw_opt():
    # experiment: let walrus hoist LDWEIGHTS (default pipeline disables it)
    import concourse.bass_utils as bu

    if getattr(bu, "_ldw_patched", False):
        return
    orig = bu.run_command

    def patched(cmd, *a, **kw):
        cmd = [
            c.replace("--enable-ldw-opt=false", "--enable-ldw-opt=true")
            if isinstance(c, str) else c
            for c in cmd
        ]
        return orig(cmd, *a, **kw)

    bu.run_command = patched
    bu._ldw_patched = True


def kernel(x, w_qkv, w_proj, b_proj):
    global LAST_RESULT
    if os.environ.get("LDW_OPT") == "1":
        _patch_ldw_opt()
    if "nc" not in _cache:
        _cache["nc"] = _build()
    nc = _cache["nc"]

    # permute qkv output columns to [q_p | k_p] head-pair-interleaved, v last
    wqkvT_full = w_qkv.astype(np.float32).T  # [c, o]
    cols = []
    for p in range(NB_C):
        cols.append(wqkvT_full[:, p * 128:(p + 1) * 128])          # q pair p
        cols.append(wqkvT_full[:, C + p * 128:C + (p + 1) * 128])  # k pair p
    cols.append(wqkvT_full[:, 2 * C:3 * C])                        # v
    wq_perm = np.concatenate(cols, axis=1).astype(BF)              # [c, 2304]
    # [c, o] -> [p, chunk, cb, 256] partition-major
    wqkvT = np.ascontiguousarray(
        wq_perm.reshape(NB_C, 128, 9, 256).transpose(1, 2, 0, 3)
    )
    wprojT = np.ascontiguousarray(
        w_proj.astype(np.float32).T.astype(BF).reshape(NB_C, 128, C).transpose(1, 0, 2)
    )
    bias = np.ascontiguousarray(b_proj.astype(np.float32).reshape(NB_C, 128).T)
    in_maps = []
    for i in range(N_CORES):
        xT = np.ascontiguousarray(
            x[i].astype(np.float32).T.astype(BF).reshape(NB_C, 128, S).transpose(1, 0, 2)
        )
        in_maps.append({"xT": xT, "wqkvT": wqkvT, "wprojT": wprojT, "bias": bias})

    res = run_bass_kernel_spmd(
        nc, in_maps, core_ids=list(range(N_CORES)), trace=TRACE
    )
    LAST_RESULT = res

    out = np.empty((N_CORES, S, C), np.float32)
    for i in range(N_CORES):
        out[i] = res.results[i]["out"].reshape(C, S).T
    return out


# revision 24
# speedup vs baseline: 1.0611x; 1.0119x over previous
"""Multi-head attention (B=8, S=1024, C=768, H=12, Dh=64) on 8 TRN2 NeuronCores.

Strategy: pure data parallelism — batch element i runs on core i. Per core:
  qkvT = w_qkv @ x^T      (o-major layout; q,k kept transposed [d, s])
  v    = x @ w_qkv_v^T    (t-major layout, augmented with a ones column)
  scoresT[t,s] = k q^T    -> exp (no max-subtract; logits are small)
  outU^T = [v | 1]^T @ att  (row 64 of PSUM = softmax denominators)
  outT = outU^T / sums;  y^T = w_proj @ outT + b
All matmuls run in bf16 (fp32 PSUM accumulation). Host pre-transposes
x / weights so no on-chip transposes are needed; host converts to bf16.
"""
import os

import numpy as np
import ml_dtypes

import concourse.bass as bass
import concourse.tile as tile
from concourse import bacc, mybir
from concourse.bass_utils import run_bass_kernel_spmd

N_CORES = 8
S, C, H, Dh = 1024, 768, 12, 64
NB_C = C // 128          # 6 c-blocks
NB_S = S // 128          # 8 s/t-blocks
NJ = S // 512            # 2 free-dim column blocks
f32 = mybir.dt.float32
bf16 = mybir.dt.bfloat16
BF = ml_dtypes.bfloat16

TRACE = False
LAST_RESULT = None

_cache = {}


def _build():
    nc = bacc.Bacc("TRN2", target_bir_lowering=False, debug=False, num_devices=N_CORES)
    xT_d = nc.dram_tensor("xT", [128, NJ, NB_C, 512], bf16, kind="ExternalInput").ap()
    wq_d = nc.dram_tensor("wqkvT", [128, 9, NB_C, 256], bf16, kind="ExternalInput").ap()
    wp_d = nc.dram_tensor("wprojT", [128, NB_C, C], bf16, kind="ExternalInput").ap()
    bias_d = nc.dram_tensor("bias", [128, NB_C], f32, kind="ExternalInput").ap()
    out_d = nc.dram_tensor("out", [NB_C, 128, S], f32, kind="ExternalOutput").ap()

    Exp = mybir.ActivationFunctionType.Exp

    with tile.TileContext(nc) as tc:
        with (
            tc.tile_pool(name="singles", bufs=1) as singles,
            tc.tile_pool(name="att", bufs=16) as att_pool,
            tc.tile_pool(name="inv", bufs=4) as inv_pool,
            tc.tile_pool(name="binv", bufs=4) as binv_pool,
            tc.tile_pool(name="ya", bufs=12) as ya_pool,
            tc.tile_pool(name="y", bufs=3) as y_pool,
            tc.tile_pool(name="qkv_ps", bufs=2, space="PSUM") as qkv_ps,
            tc.tile_pool(name="sc_ps", bufs=2, space="PSUM") as sc_ps,
            tc.tile_pool(name="av_ps", bufs=2, space="PSUM") as av_ps,
        ):
            xT = singles.tile([128, NB_C, S], bf16)
            wq = singles.tile([128, NB_C, 3 * C], bf16)
            wp = singles.tile([128, NB_C, C], bf16)
            bias = singles.tile([128, NB_C], f32)
            # DMA order = dependency order: xT, q/k pair 0, v, pairs 1-5,
            # wp, bias. Host layouts are partition-major so every transfer is
            # ONE dma_start with large per-partition-contiguous descriptors.
            nc.sync.dma_start(out=xT[:, :, 0:512], in_=xT_d[:, 0])
            nc.sync.dma_start(out=wq[:, :, 0:256], in_=wq_d[:, 0])
            nc.sync.dma_start(out=xT[:, :, 512:1024], in_=xT_d[:, 1])
            for c3 in range(3):
                nc.sync.dma_start(
                    out=wq[:, :, 2 * C + c3 * 256:2 * C + (c3 + 1) * 256],
                    in_=wq_d[:, 6 + c3],
                )
            for p in range(1, 6):
                nc.sync.dma_start(out=wq[:, :, p * 256:(p + 1) * 256], in_=wq_d[:, p])
            nc.sync.dma_start(out=wp[:], in_=wp_d[:])
            nc.sync.dma_start(out=bias[:], in_=bias_d[:])

            qkT = singles.tile([128, 2 * NB_C, S], bf16)   # q at idx p, k at idx 6+p
            vaug = singles.tile([128, NB_S, H, 65], bf16)  # [t, tb, h, d|1]
            outUT = singles.tile([128, NB_C, S], bf16)

            # PE warmup during the DMA fill: dummy matmuls with NO deps at all
            # (uninitialized SBUF is fine - the psum result is never read).
            # PE starts the moment the engine comes up and un-throttles the
            # HAM clock gate before the first real matmul.
            warm = singles.tile([128, 512], bf16)
            nc.vector.memset(warm[:], 0.0)
            for wi in range(2):
                ps_w = qkv_ps.tile([128, 512], f32, tag="qkvps")
                for wj in range(8):
                    nc.tensor.matmul(ps_w[:], warm[:, 0:128], warm[:], start=True, stop=True)

            # ones column of v_aug: memset the WHOLE tile to 1.0 (contiguous,
            # fast, on the idle gpsimd) - A-v copies overwrite cols 0-63 later.
            nc.gpsimd.memset(vaug[:], 1.0)

            # ---------- emission helpers ----------
            def emit_aqk(p, which, j):
                # q/k projection: head-pair p, which 0=q 1=k, s-column block j
                ps_qk = qkv_ps.tile([128, 512], f32, tag="qkvps")
                col0 = p * 256 + which * 128
                for cb in range(NB_C):
                    nc.tensor.matmul(
                        ps_qk[:],
                        wq[:, cb, col0:col0 + 128],
                        xT[:, cb, j * 512:(j + 1) * 512],
                        start=(cb == 0), stop=(cb == NB_C - 1),
                    )
                nc.vector.tensor_copy(
                    qkT[:, which * NB_C + p, j * 512:(j + 1) * 512], ps_qk[:]
                )

            def emit_av_stage(tb, half):
                # v projection into the augmented [t, h, d|1] tile
                ps_v = qkv_ps.tile([128, 512], f32, tag="qkvps")
                for cb in range(NB_C):
                    nc.tensor.matmul(
                        ps_v[:, 0:384],
                        xT[:, cb, tb * 128:(tb + 1) * 128],
                        wq[:, cb, 2 * C + half * 384:2 * C + (half + 1) * 384],
                        start=(cb == 0), stop=(cb == NB_C - 1),
                    )
                nc.vector.tensor_copy(
                    vaug[:, tb, half * 6:(half + 1) * 6, 0:64],
                    ps_v[:, 0:384].rearrange("p (h d) -> p h d", d=64),
                )

            atts = [[None] * NB_S for _ in range(H)]
            pavs = [[None] * NJ for _ in range(H)]
            sums_t = [None] * H

            def emit_qk(h, tb):
                hp = 64 * (h % 2)
                p = h // 2
                qT = qkT[hp:hp + 64, p, :]
                kT = qkT[hp:hp + 64, NB_C + p, :]
                att = att_pool.tile([128, S], bf16, tag="att")
                atts[h][tb] = att
                sc = sc_ps.tile([128, 1024], f32, tag="scps")
                for j in range(NJ):
                    nc.tensor.matmul(
                        sc[:, j * 512:(j + 1) * 512],
                        kT[:, tb * 128:(tb + 1) * 128],
                        qT[:, j * 512:(j + 1) * 512],
                        start=True, stop=True,
                    )
                nc.scalar.activation(att[:], sc[:], Exp, scale=0.125)

            def emit_av(h, j):
                if sums_t[h] is None:
                    sums_t[h] = inv_pool.tile([1, S], f32, tag="sums", name=f"sums{h}")
                pav = av_ps.tile([65, 512], f32, tag="avps")
                pavs[h][j] = pav
                for tb in range(NB_S):
                    nc.tensor.matmul(
                        pav[:],
                        vaug[:, tb, h, :],
                        atts[h][tb][:, j * 512:(j + 1) * 512],
                        start=(tb == 0), stop=(tb == NB_S - 1),
                    )
                nc.vector.tensor_copy(
                    sums_t[h][:, j * 512:(j + 1) * 512], pav[64:65, :]
                )

            def emit_norm_j(h, j):
                hp = 64 * (h % 2)
                inv = inv_pool.tile([1, 512], f32, tag="inv", name=f"invj{h}_{j}")
                nc.vector.reciprocal_approx_fast(
                    inv[:], sums_t[h][:, j * 512:(j + 1) * 512]
                )
                binv = binv_pool.tile([128, 512], f32, tag="binv", name=f"binvj{h}_{j}")
                nc.gpsimd.partition_broadcast(binv[:], inv[:])
                nc.vector.tensor_mul(
                    outUT[hp:hp + 64, h // 2, j * 512:(j + 1) * 512],
                    pavs[h][j][0:64, :],
                    binv[hp:hp + 64, :],
                )

            def emit_norm(h):
                hp = 64 * (h % 2)
                inv = inv_pool.tile([1, S], f32, tag="inv")
                nc.vector.reciprocal_approx_fast(inv[:], sums_t[h][:])
                binv = binv_pool.tile([128, S], f32, tag="binv")
                nc.gpsimd.partition_broadcast(binv[:], inv[:])
                for j in range(NJ):
                    nc.vector.tensor_mul(
                        outUT[hp:hp + 64, h // 2, j * 512:(j + 1) * 512],
                        pavs[h][j][0:64, :],
                        binv[hp:hp + 64, j * 512:(j + 1) * 512],
                    )

            ya_t = {}

            def emit_proj_a(cb, j):
                # first half-accumulation (kb 0-2 = heads 0-5) + bias
                pp = qkv_ps.tile([128, 512], f32, tag="qkvps")
                for kb in range(3):
                    nc.tensor.matmul(
                        pp[:],
                        wp[:, kb, cb * 128:(cb + 1) * 128],
                        outUT[:, kb, j * 512:(j + 1) * 512],
                        start=(kb == 0), stop=(kb == 2),
                    )
                ya = ya_pool.tile([128, 512], f32, tag="ya")
                ya_t[(cb, j)] = ya
                nc.vector.tensor_scalar_add(ya[:], pp[:], bias[:, cb:cb + 1])

            def emit_proj_b(cb, j):
                # second partial: kb 3-4 (heads 6-9), accumulate into ya
                pp = qkv_ps.tile([128, 512], f32, tag="qkvps")
                for kb in range(3, 5):
                    nc.tensor.matmul(
                        pp[:],
                        wp[:, kb, cb * 128:(cb + 1) * 128],
                        outUT[:, kb, j * 512:(j + 1) * 512],
                        start=(kb == 3), stop=(kb == 4),
                    )
                nc.vector.tensor_add(ya_t[(cb, j)][:], pp[:], ya_t[(cb, j)][:])

            def emit_proj_c(cb, j, alt=False):
                # final partial: kb 5 (heads 10-11) + combine + store
                if alt:
                    pp_t = sc_ps.tile([128, 1024], f32, tag="scps", name=f"ppc{cb}_{j}")
                    pp = pp_t[:, 0:512]
                else:
                    pp = qkv_ps.tile([128, 512], f32, tag="qkvps")
                nc.tensor.matmul(
                    pp[:],
                    wp[:, 5, cb * 128:(cb + 1) * 128],
                    outUT[:, 5, j * 512:(j + 1) * 512],
                    start=True, stop=True,
                )
                y = y_pool.tile([128, 512], f32, tag="y")
                nc.vector.tensor_add(y[:], pp[:] if not alt else pp, ya_t[(cb, j)][:])
                nc.sync.dma_start(out=out_d[cb, :, j * 512:(j + 1) * 512], in_=y[:])

            # ---------- emission order ----------
            # bootstrap: pair 0 q/k, then head-0 QKs interleaved with A-v half 0
            for j in range(NJ):
                emit_aqk(0, 0, j)
            for j in range(NJ):
                emit_aqk(0, 1, j)
            emit_qk(0, 0)
            emit_qk(0, 1)
            counts = [2, 2, 1, 1, 1, 1]
            ui = 0
            for i, tb in enumerate(range(2, NB_S)):
                for _ in range(counts[i]):
                    emit_av_stage(ui, 0)
                    ui += 1
                emit_qk(0, tb)

            # filler units per head period: A-v half1, remaining q/k pairs,
            # then the first two projection partials
            fillers = {
                1: [("aqk", 1), ("av", 0), ("av", 1)],
                2: [("aqk", 2), ("av", 2), ("av", 3)],
                3: [("aqk", 3), ("av", 4), ("av", 5)],
                4: [("av", 6), ("av", 7)],
                5: [("aqk", 4)],
                6: [("pa", 0), ("pa", 1), ("pa", 2), ("pa", 3)],
                7: [("aqk", 5)],
                8: [("pa", 4), ("pa", 5), ("pa", 6), ("pa", 7)],
                9: [("pa", 8), ("pa", 9), ("pa", 10), ("pa", 11)],
                10: [("pb", 0), ("pb", 1), ("pb", 2), ("pb", 3), ("pb", 4)],
                11: [("pb", 5), ("pb", 6), ("pb", 7), ("pb", 8)],
            }
            cj_units = [(cb, j) for cb in range(NB_C) for j in range(NJ)]

            def emit_filler(kind, a):
                if kind == "aqk":
                    for j in range(NJ):
                        emit_aqk(a, 0, j)
                        emit_aqk(a, 1, j)
                elif kind == "av":
                    # A-v half 1, tb index a (half 0 done in bootstrap)
                    emit_av_stage(a, 1)
                elif kind == "pa":
                    emit_proj_a(*cj_units[a])
                elif kind == "pb":
                    emit_proj_b(*cj_units[a])

            for h in range(1, H):
                fl = list(fillers.get(h, []))
                emit_qk(h, 0)
                emit_qk(h, 1)
                emit_av(h - 1, 0)
                emit_qk(h, 2)
                if fl and fl[0][0] in ("aqk", "av"):
                    emit_filler(*fl.pop(0))
                emit_qk(h, 3)
                emit_av(h - 1, 1)
                emit_norm(h - 1)
                emit_qk(h, 4)
                if fl:
                    emit_filler(*fl.pop(0))
                emit_qk(h, 5)
                emit_qk(h, 6)
                for u in fl:
                    emit_filler(*u)
                emit_qk(h, 7)
            emit_av(H - 1, 0)
            emit_norm_j(H - 1, 0)
            emit_proj_b(*cj_units[9])
            emit_av(H - 1, 1)
            emit_norm_j(H - 1, 1)
            emit_proj_b(*cj_units[10])
            emit_proj_b(*cj_units[11])
            for j in range(NJ):
                for cb in range(NB_C):
                    emit_proj_c(cb, j, alt=(cb % 2 == 1))

    nc.compile()
    return nc


def _patch_ldw_opt():
    # experiment: let walrus hoist LDWEIGHTS (default pipeline disables it)
    import concourse.bass_utils as bu

    if getattr(bu, "_ldw_patched", False):
        return
    orig = bu.run_command

    def patched(cmd, *a, **kw):
        cmd = [
            c.replace("--enable-ldw-opt=false", "--enable-ldw-opt=true")
            if isinstance(c, str) else c
            for c in cmd
        ]
        return orig(cmd, *a, **kw)

    bu.run_command = patched
    bu._ldw_patched = True


def kernel(x, w_qkv, w_proj, b_proj):
    global LAST_RESULT
    if os.environ.get("LDW_OPT") == "1":
        _patch_ldw_opt()
    if "nc" not in _cache:
        _cache["nc"] = _build()
    nc = _cache["nc"]

    # permute qkv output columns to [q_p | k_p] head-pair-interleaved, v last
    wqkvT_full = w_qkv.astype(np.float32).T  # [c, o]
    cols = []
    for p in range(NB_C):
        cols.append(wqkvT_full[:, p * 128:(p + 1) * 128])          # q pair p
        cols.append(wqkvT_full[:, C + p * 128:C + (p + 1) * 128])  # k pair p
    cols.append(wqkvT_full[:, 2 * C:3 * C])                        # v
    wq_perm = np.concatenate(cols, axis=1).astype(BF)              # [c, 2304]
    # [c, o] -> [p, chunk, cb, 256] partition-major
    wqkvT = np.ascontiguousarray(
        wq_perm.reshape(NB_C, 128, 9, 256).transpose(1, 2, 0, 3)
    )
    wprojT = np.ascontiguousarray(
        w_proj.astype(np.float32).T.astype(BF).reshape(NB_C, 128, C).transpose(1, 0, 2)
    )
    bias = np.ascontiguousarray(b_proj.astype(np.float32).reshape(NB_C, 128).T)
    in_maps = []
    for i in range(N_CORES):
        xT = np.ascontiguousarray(
            x[i].astype(np.float32).T.astype(BF)
            .reshape(NB_C, 128, NJ, 512).transpose(1, 2, 0, 3)
        )
        in_maps.append({"xT": xT, "wqkvT": wqkvT, "wprojT": wprojT, "bias": bias})

    res = run_bass_kernel_spmd(
        nc, in_maps, core_ids=list(range(N_CORES)), trace=TRACE
    )
    LAST_RESULT = res

    out = np.empty((N_CORES, S, C), np.float32)
    for i in range(N_CORES):
        out[i] = res.results[i]["out"].reshape(C, S).T
    return out


# revision 26
# speedup vs baseline: 1.0652x; 1.0039x over previous
"""Multi-head attention (B=8, S=1024, C=768, H=12, Dh=64) on 8 TRN2 NeuronCores.

Strategy: pure data parallelism — batch element i runs on core i. Per core:
  qkvT = w_qkv @ x^T      (o-major layout; q,k kept transposed [d, s])
  v    = x @ w_qkv_v^T    (t-major layout, augmented with a ones column)
  scoresT[t,s] = k q^T    -> exp (no max-subtract; logits are small)
  outU^T = [v | 1]^T @ att  (row 64 of PSUM = softmax denominators)
  outT = outU^T / sums;  y^T = w_proj @ outT + b
All matmuls run in bf16 (fp32 PSUM accumulation). Host pre-transposes
x / weights so no on-chip transposes are needed; host converts to bf16.
"""
import os

import numpy as np
import ml_dtypes

import concourse.bass as bass
import concourse.tile as tile
from concourse import bacc, mybir
from concourse.bass_utils import run_bass_kernel_spmd

N_CORES = 8
S, C, H, Dh = 1024, 768, 12, 64
NB_C = C // 128          # 6 c-blocks
NB_S = S // 128          # 8 s/t-blocks
NJ = S // 512            # 2 free-dim column blocks
f32 = mybir.dt.float32
bf16 = mybir.dt.bfloat16
BF = ml_dtypes.bfloat16

TRACE = False
LAST_RESULT = None

_cache = {}


def _build():
    nc = bacc.Bacc("TRN2", target_bir_lowering=False, debug=False, num_devices=N_CORES)
    xT_d = nc.dram_tensor("xT", [128, NJ, NB_C, 512], bf16, kind="ExternalInput").ap()
    wq_d = nc.dram_tensor("wqkvT", [128, 9, NB_C, 256], bf16, kind="ExternalInput").ap()
    wp_d = nc.dram_tensor("wprojT", [128, NB_C, C], bf16, kind="ExternalInput").ap()
    bias_d = nc.dram_tensor("bias", [128, NB_C], f32, kind="ExternalInput").ap()
    out_d = nc.dram_tensor("out", [NB_C, 128, S], f32, kind="ExternalOutput").ap()

    Exp = mybir.ActivationFunctionType.Exp

    with tile.TileContext(nc) as tc:
        with (
            tc.tile_pool(name="singles", bufs=1) as singles,
            tc.tile_pool(name="att", bufs=16) as att_pool,
            tc.tile_pool(name="inv", bufs=4) as inv_pool,
            tc.tile_pool(name="binv", bufs=4) as binv_pool,
            tc.tile_pool(name="ya", bufs=12) as ya_pool,
            tc.tile_pool(name="y", bufs=3) as y_pool,
            tc.tile_pool(name="qkv_ps", bufs=2, space="PSUM") as qkv_ps,
            tc.tile_pool(name="sc_ps", bufs=2, space="PSUM") as sc_ps,
            tc.tile_pool(name="av_ps", bufs=2, space="PSUM") as av_ps,
        ):
            xT = singles.tile([128, NB_C, S], bf16)
            wq = singles.tile([128, NB_C, 3 * C], bf16)
            wp = singles.tile([128, NB_C, C], bf16)
            bias = singles.tile([128, NB_C], f32)
            # DMA order = dependency order: xT, q/k pair 0, v, pairs 1-5,
            # wp, bias. Host layouts are partition-major so every transfer is
            # ONE dma_start with large per-partition-contiguous descriptors.
            nc.sync.dma_start(out=xT[:, :, 0:512], in_=xT_d[:, 0])
            nc.sync.dma_start(out=wq[:, :, 0:256], in_=wq_d[:, 0])
            nc.sync.dma_start(out=xT[:, :, 512:1024], in_=xT_d[:, 1])
            for c3 in range(3):
                nc.sync.dma_start(
                    out=wq[:, :, 2 * C + c3 * 256:2 * C + (c3 + 1) * 256],
                    in_=wq_d[:, 6 + c3],
                )
            for p in range(1, 6):
                nc.sync.dma_start(out=wq[:, :, p * 256:(p + 1) * 256], in_=wq_d[:, p])
            nc.sync.dma_start(out=wp[:], in_=wp_d[:])
            nc.sync.dma_start(out=bias[:], in_=bias_d[:])

            qkT = singles.tile([128, 2 * NB_C, S], bf16)   # q at idx p, k at idx 6+p
            vaug = singles.tile([128, NB_S, H, 65], bf16)  # [t, tb, h, d|1]
            outUT = singles.tile([128, NB_C, S], bf16)

            # PE warmup during the DMA fill: dummy matmuls with NO deps at all
            # (uninitialized SBUF is fine - the psum result is never read).
            # PE starts the moment the engine comes up and un-throttles the
            # HAM clock gate before the first real matmul.
            warm = singles.tile([128, 512], bf16)
            nc.vector.memset(warm[:], 0.0)
            for wi in range(2):
                ps_w = qkv_ps.tile([128, 512], f32, tag="qkvps")
                for wj in range(8):
                    nc.tensor.matmul(ps_w[:], warm[:, 0:128], warm[:], start=True, stop=True)

            # ones column of v_aug: memset the WHOLE tile to 1.0 (contiguous,
            # fast, on the idle gpsimd) - A-v copies overwrite cols 0-63 later.
            nc.gpsimd.memset(vaug[:], 1.0)

            # ---------- emission helpers ----------
            def emit_aqk(p, which, j):
                # q/k projection: head-pair p, which 0=q 1=k, s-column block j
                ps_qk = qkv_ps.tile([128, 512], f32, tag="qkvps")
                col0 = p * 256 + which * 128
                for cb in range(NB_C):
                    nc.tensor.matmul(
                        ps_qk[:],
                        wq[:, cb, col0:col0 + 128],
                        xT[:, cb, j * 512:(j + 1) * 512],
                        start=(cb == 0), stop=(cb == NB_C - 1),
                    )
                nc.vector.tensor_copy(
                    qkT[:, which * NB_C + p, j * 512:(j + 1) * 512], ps_qk[:]
                )

            def emit_av_stage(tb, half):
                # v projection into the augmented [t, h, d|1] tile
                ps_v = qkv_ps.tile([128, 512], f32, tag="qkvps")
                for cb in range(NB_C):
                    nc.tensor.matmul(
                        ps_v[:, 0:384],
                        xT[:, cb, tb * 128:(tb + 1) * 128],
                        wq[:, cb, 2 * C + half * 384:2 * C + (half + 1) * 384],
                        start=(cb == 0), stop=(cb == NB_C - 1),
                    )
                nc.vector.tensor_copy(
                    vaug[:, tb, half * 6:(half + 1) * 6, 0:64],
                    ps_v[:, 0:384].rearrange("p (h d) -> p h d", d=64),
                )

            atts = [[None] * NB_S for _ in range(H)]
            pavs = [[None] * NJ for _ in range(H)]
            sums_t = [None] * H

            def emit_qk(h, tb):
                hp = 64 * (h % 2)
                p = h // 2
                qT = qkT[hp:hp + 64, p, :]
                kT = qkT[hp:hp + 64, NB_C + p, :]
                att = att_pool.tile([128, S], bf16, tag="att")
                atts[h][tb] = att
                sc = sc_ps.tile([128, 1024], f32, tag="scps")
                for j in range(NJ):
                    nc.tensor.matmul(
                        sc[:, j * 512:(j + 1) * 512],
                        kT[:, tb * 128:(tb + 1) * 128],
                        qT[:, j * 512:(j + 1) * 512],
                        start=True, stop=True,
                    )
                nc.scalar.activation(att[:], sc[:], Exp, scale=0.125)

            def emit_av(h, j):
                if sums_t[h] is None:
                    sums_t[h] = inv_pool.tile([1, S], f32, tag="sums", name=f"sums{h}")
                pav = av_ps.tile([65, 512], f32, tag="avps")
                pavs[h][j] = pav
                for tb in range(NB_S):
                    nc.tensor.matmul(
                        pav[:],
                        vaug[:, tb, h, :],
                        atts[h][tb][:, j * 512:(j + 1) * 512],
                        start=(tb == 0), stop=(tb == NB_S - 1),
                    )
                nc.vector.tensor_copy(
                    sums_t[h][:, j * 512:(j + 1) * 512], pav[64:65, :]
                )

            def emit_norm_j(h, j):
                hp = 64 * (h % 2)
                inv = inv_pool.tile([1, 512], f32, tag="inv", name=f"invj{h}_{j}")
                nc.vector.reciprocal_approx_fast(
                    inv[:], sums_t[h][:, j * 512:(j + 1) * 512]
                )
                binv = binv_pool.tile([128, 512], f32, tag="binv", name=f"binvj{h}_{j}")
                nc.gpsimd.partition_broadcast(binv[:], inv[:])
                nc.vector.tensor_mul(
                    outUT[hp:hp + 64, h // 2, j * 512:(j + 1) * 512],
                    pavs[h][j][0:64, :],
                    binv[hp:hp + 64, :],
                )

            def emit_norm(h):
                hp = 64 * (h % 2)
                inv = inv_pool.tile([1, S], f32, tag="inv")
                nc.vector.reciprocal_approx_fast(inv[:], sums_t[h][:])
                binv = binv_pool.tile([128, S], f32, tag="binv")
                nc.gpsimd.partition_broadcast(binv[:], inv[:])
                for j in range(NJ):
                    nc.vector.tensor_mul(
                        outUT[hp:hp + 64, h // 2, j * 512:(j + 1) * 512],
                        pavs[h][j][0:64, :],
                        binv[hp:hp + 64, j * 512:(j + 1) * 512],
                    )

            ya_t = {}

            def emit_proj_a(cb, j):
                # first half-accumulation (kb 0-2 = heads 0-5) + bias
                pp = qkv_ps.tile([128, 512], f32, tag="qkvps")
                for kb in range(3):
                    nc.tensor.matmul(
                        pp[:],
                        wp[:, kb, cb * 128:(cb + 1) * 128],
                        outUT[:, kb, j * 512:(j + 1) * 512],
                        start=(kb == 0), stop=(kb == 2),
                    )
                ya = ya_pool.tile([128, 512], f32, tag="ya")
                ya_t[(cb, j)] = ya
                nc.vector.tensor_scalar_add(ya[:], pp[:], bias[:, cb:cb + 1])

            def emit_proj_b(cb, j):
                # second partial: kb 3-4 (heads 6-9), accumulate into ya
                pp = qkv_ps.tile([128, 512], f32, tag="qkvps")
                for kb in range(3, 5):
                    nc.tensor.matmul(
                        pp[:],
                        wp[:, kb, cb * 128:(cb + 1) * 128],
                        outUT[:, kb, j * 512:(j + 1) * 512],
                        start=(kb == 3), stop=(kb == 4),
                    )
                nc.vector.tensor_add(ya_t[(cb, j)][:], pp[:], ya_t[(cb, j)][:])

            def emit_proj_c(cb, j, alt=0):
                # final partial: kb 5 (heads 10-11) + combine + store
                if alt == 1:
                    pp_t = sc_ps.tile([128, 1024], f32, tag="scps", name=f"ppc{cb}_{j}")
                    pp = pp_t[:, 0:512]
                elif alt == 2:
                    pp_t = av_ps.tile([128, 512], f32, tag="avps", name=f"ppd{cb}_{j}")
                    pp = pp_t[:]
                else:
                    pp = qkv_ps.tile([128, 512], f32, tag="qkvps")
                nc.tensor.matmul(
                    pp,
                    wp[:, 5, cb * 128:(cb + 1) * 128],
                    outUT[:, 5, j * 512:(j + 1) * 512],
                    start=True, stop=True,
                )
                y = y_pool.tile([128, 512], f32, tag="y")
                nc.vector.tensor_add(y[:], pp, ya_t[(cb, j)][:])
                nc.sync.dma_start(out=out_d[cb, :, j * 512:(j + 1) * 512], in_=y[:])

            # ---------- emission order ----------
            # bootstrap: pair 0 q/k, then head-0 QKs interleaved with A-v half 0
            for j in range(NJ):
                emit_aqk(0, 0, j)
            for j in range(NJ):
                emit_aqk(0, 1, j)
            emit_qk(0, 0)
            emit_qk(0, 1)
            counts = [2, 2, 1, 1, 1, 1]
            ui = 0
            for i, tb in enumerate(range(2, NB_S)):
                for _ in range(counts[i]):
                    emit_av_stage(ui, 0)
                    ui += 1
                emit_qk(0, tb)

            # filler units per head period: A-v half1, remaining q/k pairs,
            # then the first two projection partials
            fillers = {
                1: [("aqk", 1), ("av", 0), ("av", 1)],
                2: [("aqk", 2), ("av", 2), ("av", 3)],
                3: [("aqk", 3), ("av", 4), ("av", 5)],
                4: [("av", 6), ("av", 7)],
                5: [("aqk", 4)],
                6: [("pa", 0), ("pa", 1), ("pa", 2), ("pa", 3)],
                7: [("aqk", 5)],
                8: [("pa", 4), ("pa", 5), ("pa", 6), ("pa", 7)],
                9: [("pa", 8), ("pa", 9), ("pa", 10), ("pa", 11)],
                10: [("pb", 0), ("pb", 1), ("pb", 2), ("pb", 3), ("pb", 4)],
                11: [("pb", 5), ("pb", 6), ("pb", 7), ("pb", 8)],
            }
            cj_units = [(cb, j) for cb in range(NB_C) for j in range(NJ)]

            def emit_filler(kind, a):
                if kind == "aqk":
                    for j in range(NJ):
                        emit_aqk(a, 0, j)
                        emit_aqk(a, 1, j)
                elif kind == "av":
                    # A-v half 1, tb index a (half 0 done in bootstrap)
                    emit_av_stage(a, 1)
                elif kind == "pa":
                    emit_proj_a(*cj_units[a])
                elif kind == "pb":
                    emit_proj_b(*cj_units[a])

            for h in range(1, H):
                fl = list(fillers.get(h, []))
                emit_qk(h, 0)
                emit_qk(h, 1)
                emit_av(h - 1, 0)
                emit_qk(h, 2)
                if fl and fl[0][0] in ("aqk", "av"):
                    emit_filler(*fl.pop(0))
                emit_qk(h, 3)
                emit_av(h - 1, 1)
                emit_norm(h - 1)
                emit_qk(h, 4)
                if fl:
                    emit_filler(*fl.pop(0))
                emit_qk(h, 5)
                emit_qk(h, 6)
                for u in fl:
                    emit_filler(*u)
                emit_qk(h, 7)
            emit_av(H - 1, 0)
            emit_norm_j(H - 1, 0)
            emit_proj_b(*cj_units[9])
            emit_proj_b(*cj_units[10])
            for cb in range(NB_C):
                emit_proj_c(cb, 0, alt=cb % 3)
            emit_av(H - 1, 1)
            emit_norm_j(H - 1, 1)
            emit_proj_b(*cj_units[11])
            for cb in range(NB_C):
                emit_proj_c(cb, 1, alt=cb % 3)

    nc.compile()
    return nc


def _patch_ldw_opt():
    # experiment: let walrus hoist LDWEIGHTS (default pipeline disables it)
    import concourse.bass_utils as bu

    if getattr(bu, "_ldw_patched", False):
        return
    orig = bu.run_command

    def patched(cmd, *a, **kw):
        cmd = [
            c.replace("--enable-ldw-opt=false", "--enable-ldw-opt=true")
            if isinstance(c, str) else c
            for c in cmd
        ]
        return orig(cmd, *a, **kw)

    bu.run_command = patched
    bu._ldw_patched = True


def kernel(x, w_qkv, w_proj, b_proj):
    global LAST_RESULT
    if os.environ.get("LDW_OPT") == "1":
        _patch_ldw_opt()
    if "nc" not in _cache:
        _cache["nc"] = _build()
    nc = _cache["nc"]

    # permute qkv output columns to [q_p | k_p] head-pair-interleaved, v last
    wqkvT_full = w_qkv.astype(np.float32).T  # [c, o]
    cols = []
    for p in range(NB_C):
        cols.append(wqkvT_full[:, p * 128:(p + 1) * 128])          # q pair p
        cols.append(wqkvT_full[:, C + p * 128:C + (p + 1) * 128])  # k pair p
    cols.append(wqkvT_full[:, 2 * C:3 * C])                        # v
    wq_perm = np.concatenate(cols, axis=1).astype(BF)              # [c, 2304]
    # [c, o] -> [p, chunk, cb, 256] partition-major
    wqkvT = np.ascontiguousarray(
        wq_perm.reshape(NB_C, 128, 9, 256).transpose(1, 2, 0, 3)
    )
    wprojT = np.ascontiguousarray(
        w_proj.astype(np.float32).T.astype(BF).reshape(NB_C, 128, C).transpose(1, 0, 2)
    )
    bias = np.ascontiguousarray(b_proj.astype(np.float32).reshape(NB_C, 128).T)
    in_maps = []
    for i in range(N_CORES):
        xT = np.ascontiguousarray(
            x[i].astype(np.float32).T.astype(BF)
            .reshape(NB_C, 128, NJ, 512).transpose(1, 2, 0, 3)
        )
        in_maps.append({"xT": xT, "wqkvT": wqkvT, "wprojT": wprojT, "bias": bias})

    res = run_bass_kernel_spmd(
        nc, in_maps, core_ids=list(range(N_CORES)), trace=TRACE
    )
    LAST_RESULT = res

    out = np.empty((N_CORES, S, C), np.float32)
    for i in range(N_CORES):
        out[i] = res.results[i]["out"].reshape(C, S).T
    return out
